# revision 3
# baseline (speedup 1.0000x reference)
"""Grouped-dequant GEMM (y = x @ (W * group_scales)^T + bias) on 8 TRN2 NeuronCores.

Tensor-parallel (column) sharding: each core owns O/8 = 512 output features.
x is replicated; weight/scales/bias are sharded along out_features; output
shards are concatenated on the host. All FLOPs (dequant multiply, GEMM, bias
add) run on device; the host only does sharding + layout transforms.

Self-contained: hardcodes shapes from the problem spec.
  x      (4, 2048, 4096) fp16
  weight (4096, 4096)    fp16
  scales (4096, 32)      fp16   group size g=128 along in_features
  bias   (4096,)         fp16
  types  (64, 32)        int32  (unused by the exact-dequant reference math)
"""

import sys
import types as _types

sys.path.insert(0, "/opt/trn_rl_repo")


def _install_ntff_hook_shim():
    """antenv.axon_hooks is missing in this image; register the NTFF profile
    hook from trn_agent_boot so run_bass_kernel_spmd(trace=True) works."""
    if "antenv.axon_hooks" in sys.modules:
        return
    mod = _types.ModuleType("antenv.axon_hooks")
    try:
        import trn_agent_boot.trn_boot as tb

        hook = tb._ntff_profile_via_ctypes("/opt/axon/libaxon_pjrt.so")
    except Exception:
        hook = None
    mod.get_axon_ntff_profile_hook = lambda: hook
    mod.set_axon_ntff_profile_hook = lambda h: None
    sys.modules["antenv.axon_hooks"] = mod


_install_ntff_hook_shim()

import numpy as np

import concourse.bacc as bacc
import concourse.bass as bass
import concourse.mybir as mybir
import concourse.tile as tile
from concourse.bass import ds, ts
from concourse.bass_utils import run_bass_kernel_spmd
from concourse.kernels.tile_matmul import (
    composable_matmul_tile_kernel,
    dma_from_dram_kxm,
    dma_from_dram_kxn,
    dma_to_dram_mxn,
)

B, S, I, O, G = 4, 2048, 4096, 4096, 128
N_CORES = 8
OC = O // N_CORES  # 512 output features per core
M = B * S  # 8192 tokens
P = 128

_cached_nc = None


def _build_bass():
    """Build + compile the per-core Bass program (same graph on all 8 cores).

    Computes y = xT.T @ w_deqT + bias where
      kxm = xT   [I, M]  (streamed; stationary operand of the matmuls)
      kxn = wT   [I, OC] (dequantized in SBUF on load, then resident)
      out = y    [M, OC]
    """
    global _cached_nc
    if _cached_nc is not None:
        return _cached_nc

    nc = bacc.Bacc(
        "TRN2", target_bir_lowering=False, debug=False, num_devices=N_CORES
    )
    f16, f32 = mybir.dt.float16, mybir.dt.float32

    xT = nc.dram_tensor("xT", [I, M], f16, kind="ExternalInput").ap()
    wT = nc.dram_tensor("wT", [I, OC], f16, kind="ExternalInput").ap()
    srT = nc.dram_tensor("srT", [I, OC], f16, kind="ExternalInput").ap()
    bias_rep = nc.dram_tensor("bias_rep", [P, OC], f32, kind="ExternalInput").ap()
    y = nc.dram_tensor("y", [M, OC], f16, kind="ExternalOutput").ap()

    with tile.TileContext(nc) as tc:
        from contextlib import ExitStack

        with ExitStack() as ctx:
            kxm_pool = ctx.enter_context(tc.tile_pool(name="kxm_pool", bufs=48))
            kxn_pool = ctx.enter_context(tc.tile_pool(name="kxn_pool", bufs=33))
            sdeq_pool = ctx.enter_context(tc.tile_pool(name="sdeq", bufs=2))
            const_pool = ctx.enter_context(tc.tile_pool(name="const", bufs=1))

            bias_sb = const_pool.tile([P, OC], f32)
            nc.sync.dma_start(bias_sb[:], bias_rep[:, :])

            kxm_producer, kxm_shape = dma_from_dram_kxm(kxm_pool, xT)
            kxn_producer0, kxn_shape = dma_from_dram_kxn(kxn_pool, wT)

            srT_tiled = srT.rearrange("(po pi) f -> pi po f", pi=P)

            def kxn_producer(nc, md):
                # Load the weight tile, then dequantize in place:
                # w_deq[i, o] = w[i, o] * scales[o, i // G] (G == P so each
                # k-subtile shares one scale group; srT is the host-side
                # [I, OC] replication of scales^T, so this is elementwise).
                t = kxn_producer0(nc, md)
                s = sdeq_pool.tile([P, md.k_subtiles, md.n_tile], mybir.dt.float16)
                nc.sync.dma_start(
                    s[:],
                    srT_tiled[
                        :,
                        ts(md.k_tile_idx, md.k_subtiles),
                        ds(md.n_tile_idx * md.n_tile, md.n_tile),
                    ],
                )
                nc.vector.tensor_mul(t[:], t[:], s[:])
                return t

            def bias_reducer(nc, psum, sbuf, md):
                # sbuf(fp16) = psum(fp32) + bias(fp32), fused cast on DVE.
                n0 = md.n_tile_idx * md.n_tile + md.n_subtile_idx * md.n_subtile
                nc.vector.tensor_tensor(
                    sbuf,
                    psum,
                    bias_sb[:, ds(n0, md.n_subtile_slice_size)],
                    mybir.AluOpType.add,
                )

            mxn_consumer = dma_to_dram_mxn(y)

            composable_matmul_tile_kernel(
                tc=tc,
                kxm_shape=kxm_shape,
                kxn_shape=kxn_shape,
                output_type=mybir.dt.float16,
                kxm_producer=kxm_producer,
                kxn_producer=kxn_producer,
                mxn_consumer=mxn_consumer,
                mxn_subtile_reducer=bias_reducer,
                psum_n_bufs=2,
                cache_tiles=True,
                MAX_K_TILE_SIZE=128,
            )

    nc.compile()
    _cached_nc = nc
    return nc


def kernel(x, weight, scales, bias, types, g, _want_exec_time=False):
    assert int(g) == G
    x = np.asarray(x)
    weight = np.asarray(weight)
    scales = np.asarray(scales)
    bias = np.asarray(bias)
    assert x.shape == (B, S, I) and weight.shape == (O, I)

    nc = _build_bass()

    # Host-side layout: transposes + per-core shards (no math here).
    xT = np.ascontiguousarray(x.reshape(M, I).T)  # [I, M] fp16
    wT = np.ascontiguousarray(weight.T)  # [I, O] fp16
    # scales^T replicated over each group of G input rows -> [I, O]
    srT = np.ascontiguousarray(np.repeat(scales, G, axis=1).T)
    bias_rep = np.broadcast_to(
        bias.astype(np.float32)[None, :], (P, O)
    )  # [128, O] fp32

    in_maps = []
    for c in range(N_CORES):
        sl = slice(c * OC, (c + 1) * OC)
        in_maps.append(
            {
                "xT": xT,
                "wT": np.ascontiguousarray(wT[:, sl]),
                "srT": np.ascontiguousarray(srT[:, sl]),
                "bias_rep": np.ascontiguousarray(bias_rep[:, sl]),
            }
        )

    res = run_bass_kernel_spmd(
        nc, in_maps, core_ids=list(range(N_CORES)), trace=_want_exec_time
    )

    y = np.empty((M, O), dtype=np.float16)
    for c in range(N_CORES):
        y[:, c * OC : (c + 1) * OC] = res.results[c]["y"]
    out = y.reshape(B, S, O)
    if _want_exec_time:
        return out, res.exec_time_ns
    return out


# revision 9
# speedup vs baseline: 1.0428x; 1.0428x over previous
"""Grouped-dequant GEMM (y = x @ (W * group_scales)^T + bias) on 8 TRN2 NeuronCores.

Tensor-parallel (column) sharding: each core owns O/8 = 512 output features.
x is replicated; weight/scales/bias are sharded along out_features; output
shards are concatenated on the host. All FLOPs (dequant multiply, GEMM, bias
add) run on device; the host only does sharding + layout transforms.

Self-contained: hardcodes shapes from the problem spec.
  x      (4, 2048, 4096) fp16
  weight (4096, 4096)    fp16
  scales (4096, 32)      fp16   group size g=128 along in_features
  bias   (4096,)         fp16
  types  (64, 32)        int32  (unused by the exact-dequant reference math)
"""

import sys
import types as _types

sys.path.insert(0, "/opt/trn_rl_repo")


def _install_ntff_hook_shim():
    """antenv.axon_hooks is missing in this image; register the NTFF profile
    hook from trn_agent_boot so run_bass_kernel_spmd(trace=True) works."""
    if "antenv.axon_hooks" in sys.modules:
        return
    mod = _types.ModuleType("antenv.axon_hooks")
    try:
        import trn_agent_boot.trn_boot as tb

        hook = tb._ntff_profile_via_ctypes("/opt/axon/libaxon_pjrt.so")
    except Exception:
        hook = None
    mod.get_axon_ntff_profile_hook = lambda: hook
    mod.set_axon_ntff_profile_hook = lambda h: None
    sys.modules["antenv.axon_hooks"] = mod


_install_ntff_hook_shim()

import numpy as np

import concourse.bacc as bacc
import concourse.bass as bass
import concourse.mybir as mybir
import concourse.tile as tile
from concourse.bass import ds, ts
from concourse.bass_utils import run_bass_kernel_spmd
from concourse.kernels.tile_matmul import (
    composable_matmul_tile_kernel,
    dma_from_dram_kxm,
    dma_from_dram_kxn,
    dma_to_dram_mxn,
)

B, S, I, O, G = 4, 2048, 4096, 4096, 128
N_CORES = 8
OC = O // N_CORES  # 512 output features per core
M = B * S  # 8192 tokens
P = 128

_cached_nc = None


def _build_bass():
    """Build + compile the per-core Bass program (same graph on all 8 cores).

    Computes y = xT.T @ w_deqT + bias where
      kxm = xT   [I, M]  (streamed; stationary operand of the matmuls)
      kxn = wT   [I, OC] (dequantized in SBUF on load, then resident)
      out = y    [M, OC]
    """
    global _cached_nc
    if _cached_nc is not None:
        return _cached_nc

    nc = bacc.Bacc(
        "TRN2", target_bir_lowering=False, debug=False, num_devices=N_CORES
    )
    f16, f32 = mybir.dt.float16, mybir.dt.float32

    xT = nc.dram_tensor("xT", [I, M], f16, kind="ExternalInput").ap()
    wT = nc.dram_tensor("wT", [I, OC], f16, kind="ExternalInput").ap()
    scT = nc.dram_tensor("scT", [I // G, OC], f16, kind="ExternalInput").ap()
    bias_rep = nc.dram_tensor("bias_rep", [P, OC], f32, kind="ExternalInput").ap()
    y = nc.dram_tensor("y", [M, OC], f16, kind="ExternalOutput").ap()

    with tile.TileContext(nc) as tc:
        from contextlib import ExitStack

        with ExitStack() as ctx:
            kxm_pool = ctx.enter_context(tc.tile_pool(name="kxm_pool", bufs=20))
            kxn_pool = ctx.enter_context(tc.tile_pool(name="kxn_pool", bufs=9))
            sdeq_pool = ctx.enter_context(tc.tile_pool(name="sdeq", bufs=2))
            const_pool = ctx.enter_context(tc.tile_pool(name="const", bufs=1))

            bias_sb = const_pool.tile([P, OC], f32)
            nc.sync.dma_start(bias_sb[:], bias_rep[:, :])
            # Compact per-group scales^T, flattened onto partition 0 so the
            # GpSimd broadcast source AP is partition-0-based (BIR verifier
            # rejects nonzero base partitions).
            scales_sb = const_pool.tile([1, (I // G) * OC], f16)
            nc.sync.dma_start(scales_sb[:], scT.rearrange("g f -> (g f)")[None, :])

            kxm_producer, kxm_shape = dma_from_dram_kxm(kxm_pool, xT)
            kxn_producer0, kxn_shape = dma_from_dram_kxn(kxn_pool, wT)

            def kxn_producer(nc, md):
                # Load the weight tile, then dequantize in place:
                # w_deq[i, o] = w[i, o] * scales[o, i // G]. G == P, so each
                # k-subtile shares one scale group: broadcast that group's
                # row across partitions on GpSimd, then one DVE multiply.
                t = kxn_producer0(nc, md)
                s = sdeq_pool.tile([P, md.k_subtiles, md.n_tile], mybir.dt.float16)
                for ks in range(md.k_subtiles):
                    grp = md.k_tile_idx * md.k_subtiles + ks
                    nc.gpsimd.partition_broadcast(
                        s[:, ks, :],
                        scales_sb[
                            :, ds(grp * OC + md.n_tile_idx * md.n_tile, md.n_tile)
                        ],
                    )
                nc.vector.tensor_mul(t[:], t[:], s[:])
                return t

            def bias_reducer(nc, psum, sbuf, md):
                # sbuf(fp16) = psum(fp32) + bias(fp32), fused cast on DVE.
                n0 = md.n_tile_idx * md.n_tile + md.n_subtile_idx * md.n_subtile
                nc.vector.tensor_tensor(
                    sbuf,
                    psum,
                    bias_sb[:, ds(n0, md.n_subtile_slice_size)],
                    mybir.AluOpType.add,
                )

            mxn_consumer = dma_to_dram_mxn(y)

            composable_matmul_tile_kernel(
                tc=tc,
                kxm_shape=kxm_shape,
                kxn_shape=kxn_shape,
                output_type=mybir.dt.float16,
                kxm_producer=kxm_producer,
                kxn_producer=kxn_producer,
                mxn_consumer=mxn_consumer,
                mxn_subtile_reducer=bias_reducer,
                psum_n_bufs=2,
                cache_tiles=True,
            )

    nc.compile()
    _cached_nc = nc
    return nc


def kernel(x, weight, scales, bias, types, g, _want_exec_time=False):
    assert int(g) == G
    x = np.asarray(x)
    weight = np.asarray(weight)
    scales = np.asarray(scales)
    bias = np.asarray(bias)
    assert x.shape == (B, S, I) and weight.shape == (O, I)

    nc = _build_bass()

    # Host-side layout: transposes + per-core shards (no math here).
    xT = np.ascontiguousarray(x.reshape(M, I).T)  # [I, M] fp16
    wT = np.ascontiguousarray(weight.T)  # [I, O] fp16
    scT = np.ascontiguousarray(scales.T)  # [I//G, O] fp16
    bias_rep = np.broadcast_to(
        bias.astype(np.float32)[None, :], (P, O)
    )  # [128, O] fp32

    in_maps = []
    for c in range(N_CORES):
        sl = slice(c * OC, (c + 1) * OC)
        in_maps.append(
            {
                "xT": xT,
                "wT": np.ascontiguousarray(wT[:, sl]),
                "scT": np.ascontiguousarray(scT[:, sl]),
                "bias_rep": np.ascontiguousarray(bias_rep[:, sl]),
            }
        )

    res = run_bass_kernel_spmd(
        nc, in_maps, core_ids=list(range(N_CORES)), trace=_want_exec_time
    )

    y = np.empty((M, O), dtype=np.float16)
    for c in range(N_CORES):
        y[:, c * OC : (c + 1) * OC] = res.results[c]["y"]
    out = y.reshape(B, S, O)
    if _want_exec_time:
        return out, res.exec_time_ns
    return out


# revision 10
# speedup vs baseline: 1.0447x; 1.0019x over previous
"""Grouped-dequant GEMM (y = x @ (W * group_scales)^T + bias) on 8 TRN2 NeuronCores.

Tensor-parallel (column) sharding: each core owns O/8 = 512 output features.
x is replicated; weight/scales/bias are sharded along out_features; output
shards are concatenated on the host. All FLOPs (dequant multiply, GEMM, bias
add) run on device; the host only does sharding + layout transforms.

Self-contained: hardcodes shapes from the problem spec.
  x      (4, 2048, 4096) fp16
  weight (4096, 4096)    fp16
  scales (4096, 32)      fp16   group size g=128 along in_features
  bias   (4096,)         fp16
  types  (64, 32)        int32  (unused by the exact-dequant reference math)
"""

import sys
import types as _types

sys.path.insert(0, "/opt/trn_rl_repo")


def _install_ntff_hook_shim():
    """antenv.axon_hooks is missing in this image; register the NTFF profile
    hook from trn_agent_boot so run_bass_kernel_spmd(trace=True) works."""
    if "antenv.axon_hooks" in sys.modules:
        return
    mod = _types.ModuleType("antenv.axon_hooks")
    try:
        import trn_agent_boot.trn_boot as tb

        hook = tb._ntff_profile_via_ctypes("/opt/axon/libaxon_pjrt.so")
    except Exception:
        hook = None
    mod.get_axon_ntff_profile_hook = lambda: hook
    mod.set_axon_ntff_profile_hook = lambda h: None
    sys.modules["antenv.axon_hooks"] = mod


_install_ntff_hook_shim()

import numpy as np

import concourse.bacc as bacc
import concourse.bass as bass
import concourse.mybir as mybir
import concourse.tile as tile
from concourse.bass import ds, ts
from concourse.bass_utils import run_bass_kernel_spmd
from concourse.kernels.tile_matmul import (
    composable_matmul_tile_kernel,
    dma_from_dram_kxm,
    dma_from_dram_kxn,
    dma_to_dram_mxn,
)

B, S, I, O, G = 4, 2048, 4096, 4096, 128
N_CORES = 8
OC = O // N_CORES  # 512 output features per core
M = B * S  # 8192 tokens
P = 128

_cached_nc = None


def _build_bass():
    """Build + compile the per-core Bass program (same graph on all 8 cores).

    Computes y = xT.T @ w_deqT + bias where
      kxm = xT   [I, M]  (streamed; stationary operand of the matmuls)
      kxn = wT   [I, OC] (dequantized in SBUF on load, then resident)
      out = y    [M, OC]
    """
    global _cached_nc
    if _cached_nc is not None:
        return _cached_nc

    nc = bacc.Bacc(
        "TRN2", target_bir_lowering=False, debug=False, num_devices=N_CORES
    )
    f16, f32 = mybir.dt.float16, mybir.dt.float32

    xT = nc.dram_tensor("xT", [I, M], f16, kind="ExternalInput").ap()
    wT = nc.dram_tensor("wT", [I, OC], f16, kind="ExternalInput").ap()
    scT = nc.dram_tensor("scT", [I // G, OC], f16, kind="ExternalInput").ap()
    bias_rep = nc.dram_tensor("bias_rep", [P, OC], f32, kind="ExternalInput").ap()
    y = nc.dram_tensor("y", [M, OC], f16, kind="ExternalOutput").ap()

    with tile.TileContext(nc) as tc:
        from contextlib import ExitStack

        with ExitStack() as ctx:
            kxm_pool = ctx.enter_context(tc.tile_pool(name="kxm_pool", bufs=20))
            kxn_pool = ctx.enter_context(tc.tile_pool(name="kxn_pool", bufs=9))
            sdeq_pool = ctx.enter_context(tc.tile_pool(name="sdeq", bufs=4))
            const_pool = ctx.enter_context(tc.tile_pool(name="const", bufs=1))

            bias_sb = const_pool.tile([P, OC], f32)
            nc.sync.dma_start(bias_sb[:], bias_rep[:, :])
            # Compact per-group scales^T, flattened onto partition 0 so the
            # GpSimd broadcast source AP is partition-0-based (BIR verifier
            # rejects nonzero base partitions).
            scales_sb = const_pool.tile([1, (I // G) * OC], f16)
            nc.sync.dma_start(scales_sb[:], scT.rearrange("g f -> (g f)")[None, :])

            kxm_producer, kxm_shape = dma_from_dram_kxm(kxm_pool, xT)
            _, kxn_shape = dma_from_dram_kxn(kxn_pool, wT)

            # Prologue: load + dequantize the whole weight shard up front.
            # Emitted before the composable kernel so these DMAs beat the
            # x-prefetch flood into the queues, and all GpSimd broadcasts
            # finish before PSUM evictions start (GpSimd shares an SBUF
            # port with DVE).  w_deq[i, o] = w[i, o] * scales[o, i // G];
            # G == P so each k-subtile shares one scale group.
            wT_tiled = wT.rearrange("(po pi) f -> pi po f", pi=P)
            K_SUB = 4  # k-subtiles per 512-deep k-tile
            K_TILES = I // (P * K_SUB)  # 8
            wdeq_tiles = []
            for k in range(K_TILES):
                t = kxn_pool.tile([P, K_SUB, OC], f16, tag="wdeq")
                nc.sync.dma_start(t[:], wT_tiled[:, ts(k, K_SUB), :])
                s = sdeq_pool.tile([P, K_SUB, OC], f16)
                nc.gpsimd.partition_broadcast(
                    s.rearrange("p a b -> p (a b)"),
                    scales_sb[:, ds(k * K_SUB * OC, K_SUB * OC)],
                )
                nc.vector.tensor_mul(t[:], t[:], s[:])
                wdeq_tiles.append(t)

            def kxn_producer(nc, md):
                assert md.k_subtiles == K_SUB
                return wdeq_tiles[md.k_tile_idx]

            def bias_reducer(nc, psum, sbuf, md):
                # sbuf(fp16) = psum(fp32) + bias(fp32), fused cast on DVE.
                n0 = md.n_tile_idx * md.n_tile + md.n_subtile_idx * md.n_subtile
                nc.vector.tensor_tensor(
                    sbuf,
                    psum,
                    bias_sb[:, ds(n0, md.n_subtile_slice_size)],
                    mybir.AluOpType.add,
                )

            mxn_consumer = dma_to_dram_mxn(y)

            composable_matmul_tile_kernel(
                tc=tc,
                kxm_shape=kxm_shape,
                kxn_shape=kxn_shape,
                output_type=mybir.dt.float16,
                kxm_producer=kxm_producer,
                kxn_producer=kxn_producer,
                mxn_consumer=mxn_consumer,
                mxn_subtile_reducer=bias_reducer,
                psum_n_bufs=2,
                cache_tiles=True,
            )

    nc.compile()
    _cached_nc = nc
    return nc


def kernel(x, weight, scales, bias, types, g, _want_exec_time=False):
    assert int(g) == G
    x = np.asarray(x)
    weight = np.asarray(weight)
    scales = np.asarray(scales)
    bias = np.asarray(bias)
    assert x.shape == (B, S, I) and weight.shape == (O, I)

    nc = _build_bass()

    # Host-side layout: transposes + per-core shards (no math here).
    xT = np.ascontiguousarray(x.reshape(M, I).T)  # [I, M] fp16
    wT = np.ascontiguousarray(weight.T)  # [I, O] fp16
    scT = np.ascontiguousarray(scales.T)  # [I//G, O] fp16
    bias_rep = np.broadcast_to(
        bias.astype(np.float32)[None, :], (P, O)
    )  # [128, O] fp32

    in_maps = []
    for c in range(N_CORES):
        sl = slice(c * OC, (c + 1) * OC)
        in_maps.append(
            {
                "xT": xT,
                "wT": np.ascontiguousarray(wT[:, sl]),
                "scT": np.ascontiguousarray(scT[:, sl]),
                "bias_rep": np.ascontiguousarray(bias_rep[:, sl]),
            }
        )

    res = run_bass_kernel_spmd(
        nc, in_maps, core_ids=list(range(N_CORES)), trace=_want_exec_time
    )

    y = np.empty((M, O), dtype=np.float16)
    for c in range(N_CORES):
        y[:, c * OC : (c + 1) * OC] = res.results[c]["y"]
    out = y.reshape(B, S, O)
    if _want_exec_time:
        return out, res.exec_time_ns
    return out


# revision 11
# speedup vs baseline: 1.0480x; 1.0032x over previous
"""Grouped-dequant GEMM (y = x @ (W * group_scales)^T + bias) on 8 TRN2 NeuronCores.

Tensor-parallel (column) sharding: each core owns O/8 = 512 output features.
x is replicated; weight/scales/bias are sharded along out_features; output
shards are concatenated on the host. All FLOPs (dequant multiply, GEMM, bias
add) run on device; the host only does sharding + layout transforms.

Self-contained: hardcodes shapes from the problem spec.
  x      (4, 2048, 4096) fp16
  weight (4096, 4096)    fp16
  scales (4096, 32)      fp16   group size g=128 along in_features
  bias   (4096,)         fp16
  types  (64, 32)        int32  (unused by the exact-dequant reference math)
"""

import sys
import types as _types

sys.path.insert(0, "/opt/trn_rl_repo")


def _install_ntff_hook_shim():
    """antenv.axon_hooks is missing in this image; register the NTFF profile
    hook from trn_agent_boot so run_bass_kernel_spmd(trace=True) works."""
    if "antenv.axon_hooks" in sys.modules:
        return
    mod = _types.ModuleType("antenv.axon_hooks")
    try:
        import trn_agent_boot.trn_boot as tb

        hook = tb._ntff_profile_via_ctypes("/opt/axon/libaxon_pjrt.so")
    except Exception:
        hook = None
    mod.get_axon_ntff_profile_hook = lambda: hook
    mod.set_axon_ntff_profile_hook = lambda h: None
    sys.modules["antenv.axon_hooks"] = mod


_install_ntff_hook_shim()

import numpy as np

import concourse.bacc as bacc
import concourse.bass as bass
import concourse.mybir as mybir
import concourse.tile as tile
from concourse.bass import ds, ts
from concourse.bass_utils import run_bass_kernel_spmd
from concourse.kernels.tile_matmul import (
    composable_matmul_tile_kernel,
    dma_from_dram_kxm,
    dma_from_dram_kxn,
    dma_to_dram_mxn,
)

B, S, I, O, G = 4, 2048, 4096, 4096, 128
N_CORES = 8
OC = O // N_CORES  # 512 output features per core
M = B * S  # 8192 tokens
P = 128

_cached_nc = None


def _build_bass():
    """Build + compile the per-core Bass program (same graph on all 8 cores).

    Computes y = xT.T @ w_deqT + bias where
      kxm = xT   [I, M]  (streamed; stationary operand of the matmuls)
      kxn = wT   [I, OC] (dequantized in SBUF on load, then resident)
      out = y    [M, OC]
    """
    global _cached_nc
    if _cached_nc is not None:
        return _cached_nc

    nc = bacc.Bacc(
        "TRN2", target_bir_lowering=False, debug=False, num_devices=N_CORES
    )
    f16, f32 = mybir.dt.float16, mybir.dt.float32

    xT = nc.dram_tensor("xT", [I, M], f16, kind="ExternalInput").ap()
    wT = nc.dram_tensor("wT", [I, OC], f16, kind="ExternalInput").ap()
    scT = nc.dram_tensor("scT", [I // G, OC], f16, kind="ExternalInput").ap()
    bias_rep = nc.dram_tensor("bias_rep", [P, OC], f32, kind="ExternalInput").ap()
    y = nc.dram_tensor("y", [M, OC], f16, kind="ExternalOutput").ap()

    with tile.TileContext(nc) as tc:
        from contextlib import ExitStack

        with ExitStack() as ctx:
            kxm_pool = ctx.enter_context(tc.tile_pool(name="kxm_pool", bufs=20))
            kxn_pool = ctx.enter_context(tc.tile_pool(name="kxn_pool", bufs=9))
            sdeq_pool = ctx.enter_context(tc.tile_pool(name="sdeq", bufs=4))
            const_pool = ctx.enter_context(tc.tile_pool(name="const", bufs=1))

            bias_sb = const_pool.tile([P, OC], f32)
            nc.sync.dma_start(bias_sb[:], bias_rep[:, :])
            # Compact per-group scales^T, flattened onto partition 0 so the
            # GpSimd broadcast source AP is partition-0-based (BIR verifier
            # rejects nonzero base partitions).
            scales_sb = const_pool.tile([1, (I // G) * OC], f16)
            nc.sync.dma_start(scales_sb[:], scT.rearrange("g f -> (g f)")[None, :])

            kxm_producer, kxm_shape = dma_from_dram_kxm(kxm_pool, xT)
            _, kxn_shape = dma_from_dram_kxn(kxn_pool, wT)

            # w_deq[i, o] = w[i, o] * scales[o, i // G]; G == P so each
            # k-subtile shares one scale group (GpSimd partition broadcast,
            # then one DVE multiply).  k=0 is dequantized in a prologue so
            # its DMAs beat the x-prefetch flood into the queues; k=1..7
            # are produced JIT so their DMAs interleave with the x stream.
            wT_tiled = wT.rearrange("(po pi) f -> pi po f", pi=P)
            K_SUB = 4  # k-subtiles per 512-deep k-tile
            K_TILES = I // (P * K_SUB)  # 8

            def make_wdeq(k):
                t = kxn_pool.tile([P, K_SUB, OC], f16, tag="wdeq")
                nc.sync.dma_start(t[:], wT_tiled[:, ts(k, K_SUB), :])
                s = sdeq_pool.tile([P, K_SUB, OC], f16)
                nc.gpsimd.partition_broadcast(
                    s.rearrange("p a b -> p (a b)"),
                    scales_sb[:, ds(k * K_SUB * OC, K_SUB * OC)],
                )
                nc.vector.tensor_mul(t[:], t[:], s[:])
                return t

            wdeq_tiles = {0: make_wdeq(0)}

            def kxn_producer(nc, md):
                assert md.k_subtiles == K_SUB
                k = md.k_tile_idx
                if k not in wdeq_tiles:
                    wdeq_tiles[k] = make_wdeq(k)
                return wdeq_tiles[k]

            def bias_reducer(nc, psum, sbuf, md):
                # sbuf(fp16) = psum(fp32) + bias(fp32), fused cast on DVE.
                n0 = md.n_tile_idx * md.n_tile + md.n_subtile_idx * md.n_subtile
                nc.vector.tensor_tensor(
                    sbuf,
                    psum,
                    bias_sb[:, ds(n0, md.n_subtile_slice_size)],
                    mybir.AluOpType.add,
                )

            mxn_consumer = dma_to_dram_mxn(y)

            composable_matmul_tile_kernel(
                tc=tc,
                kxm_shape=kxm_shape,
                kxn_shape=kxn_shape,
                output_type=mybir.dt.float16,
                kxm_producer=kxm_producer,
                kxn_producer=kxn_producer,
                mxn_consumer=mxn_consumer,
                mxn_subtile_reducer=bias_reducer,
                psum_n_bufs=2,
                cache_tiles=True,
            )

    nc.compile()
    _cached_nc = nc
    return nc


def kernel(x, weight, scales, bias, types, g, _want_exec_time=False):
    assert int(g) == G
    x = np.asarray(x)
    weight = np.asarray(weight)
    scales = np.asarray(scales)
    bias = np.asarray(bias)
    assert x.shape == (B, S, I) and weight.shape == (O, I)

    nc = _build_bass()

    # Host-side layout: transposes + per-core shards (no math here).
    xT = np.ascontiguousarray(x.reshape(M, I).T)  # [I, M] fp16
    wT = np.ascontiguousarray(weight.T)  # [I, O] fp16
    scT = np.ascontiguousarray(scales.T)  # [I//G, O] fp16
    bias_rep = np.broadcast_to(
        bias.astype(np.float32)[None, :], (P, O)
    )  # [128, O] fp32

    in_maps = []
    for c in range(N_CORES):
        sl = slice(c * OC, (c + 1) * OC)
        in_maps.append(
            {
                "xT": xT,
                "wT": np.ascontiguousarray(wT[:, sl]),
                "scT": np.ascontiguousarray(scT[:, sl]),
                "bias_rep": np.ascontiguousarray(bias_rep[:, sl]),
            }
        )

    res = run_bass_kernel_spmd(
        nc, in_maps, core_ids=list(range(N_CORES)), trace=_want_exec_time
    )

    y = np.empty((M, O), dtype=np.float16)
    for c in range(N_CORES):
        y[:, c * OC : (c + 1) * OC] = res.results[c]["y"]
    out = y.reshape(B, S, O)
    if _want_exec_time:
        return out, res.exec_time_ns
    return out
